# revision 19
# baseline (speedup 1.0000x reference)
"""MoE layer (top-2 of 8, H=1024, FFN=4096) on 8 TRN2 NeuronCores —
4-way expert-F-split for load balance.

Two quads of 4 cores; quad q serves 4 experts (experts interleaved by
sorted token count so rank-r segment sizes match across quads). Core j
of a quad holds F-rows [j*1024, (j+1)*1024) of ALL 4 of its experts
(16.8MB, same weight footprint as expert-parallel). Each core streams
the quad's full token set once per segment (4 segments, one per
expert); per-core rows = 128 * sum(S_s) with S_s = rank-r max over the
two quads. vs expert-parallel's 512*maxcount: always <=, and ~3% less
at this seed's near-balanced routing (4128 vs 4256 token-equivalents).
Host sums the 4 F-slice partials (fp16) per expert and folds b2 + the
top-2 softmax gates.

Device schedule per core: segments sequential; per segment, token
chunks of <=512 (PSUM width; chunks stay >=236 tokens — below that
matmuls go LDWEIGHTS-bound at 98ns/load). Per chunk: GEMM1 (2 slabs x
4 f-tiles x 8 k) -> gelu+b1 -> GEMM2 (8 ht x 8 accum) -> fp16 evict on
DVE -> store. x tiles rotate through 4 SBUF slots [P,8,512]; chunk 0
is a tight dedicated tile, later chunks are host-padded to the 512
slot so DMA runs are contiguous. x triggers for chunk g+4 are emitted
after chunk g's GEMM1 so the SP engine never blocks on a slot-free
wait. DMA emission order = arrival order = consumption order; the
first GEMM1 group's deps (x chunk-0 + slab-0 m0 w1) are interleaved in
~131-262KB pieces so the first real matmul starts ~10.1us (NEFF
prologue keeps DMA dead until ~8.3us; a short PE warmup covers the
p-state ramp until then). Mid-kernel output stores ride the Act HWDGE
queue (slow but idle — on the SP queue they sit FIFO behind the ~25MB
input backlog and the o4-slot WAR chain stalls the PE for 22us); the
last chunk's stores use the by-then-empty SP queue for a fast tail.

GEMMs in bf16 (fp32 4x slower; fp8 fails the 2e-2 gate per v2
measurements). Weights pre-swizzled on host to exact SBUF layout.
"""

import os

os.environ.setdefault("NEURON_RT_RESET_CORES", "1")

import ml_dtypes
import numpy as np

import concourse.bass as bass  # noqa: F401  (bass types via bacc)
import concourse.mybir as mybir
from concourse import bacc
from concourse.tile import TileContext
from concourse.bass_utils import run_bass_kernel_spmd

H = 1024
E = 8
F = 4096
TOPK = 2
P = 128
N_CORES = 8
FP32 = mybir.dt.float32
FP16 = mybir.dt.float16
BF16 = mybir.dt.bfloat16

NSEG = 4           # experts per quad / segments per core
NSPLIT = 4         # cores sharing each expert's F dim
FL = F // NSPLIT   # 1024 F rows per core per expert
NTHL = FL // 512   # 2 slabs (of 512 F) per expert per core
MF = 512 // P      # 4 f-tiles per slab
KH = H // P        # 8 contraction tiles for GEMM1
HT = H // P        # 8 output H-tiles for GEMM2
NSL = NSEG * NTHL  # 8 slabs total per core
CKS = 512          # x slot width (PSUM-limited chunk cap)

WS1 = KH * 512               # 4096 w1 cols per slab
WSL = WS1 + MF * H           # + 4096 w2 cols

# PE warmup matmuls (128 rows each): cover the p-state ramp until the
# first real GEMM1 group's DMA deps land (~10us).
N_WARMUP = 12

_cache: dict = {}

TRACE = False
LAST_EXEC_TIME_NS = None


def _chunks(C: int):
    """Near-even token chunks, multiples of 8, each <=512."""
    nch = -(-C // CKS)
    u = C // 8
    units = [u // nch + (1 if i < u % nch else 0) for i in range(nch)]
    widths = [un * 8 for un in units]
    assert sum(widths) == C and all(0 < w <= CKS for w in widths)
    out = []
    off = 0
    for w in widths:
        out.append((off, w))
        off += w
    return out


def _build(segs: tuple):
    """Per-core program: NSEG sequential expert segments of sizes segs."""
    assert all(s % 8 == 0 and s > 0 for s in segs)
    seg_cbs = [_chunks(s) for s in segs]
    seg_off = [sum(segs[:s]) for s in range(NSEG)]
    CTOT = sum(segs)
    # global chunk list: (seg, chunk-in-seg, out col offset, width)
    chunks = []
    for s in range(NSEG):
        for ci, (coff, ck) in enumerate(seg_cbs[s]):
            chunks.append((s, ci, seg_off[s] + coff, ck))
    NCH = len(chunks)

    nc = bacc.Bacc("TRN2", target_bir_lowering=False, debug=False,
                   num_devices=N_CORES)

    ck0 = chunks[0][3]
    wall = nc.dram_tensor("wall", [NSL * P, WSL], BF16, kind="ExternalInput")
    # x: chunk 0 tight ([P, KH*ck0], cols k*ck0 + c) so the start-window
    # transfer is minimal; chunks 1+ are [P, KH*CKS] blocks (cols
    # k*CKS + c, zero-padded past ck) so slot DMA is fully contiguous
    xc = nc.dram_tensor("xc", [P, KH * ck0 + (NCH - 1) * KH * CKS], BF16,
                        kind="ExternalInput")
    b1c = nc.dram_tensor("b1c", [P, NSEG * FL // P], FP32,
                         kind="ExternalInput")
    out = nc.dram_tensor("out", [H, CTOT], FP16, kind="ExternalOutput")

    out_v = out.rearrange("(t p) c -> p t c", p=P)   # [128, 8, CTOT]

    GELU = mybir.ActivationFunctionType.Gelu

    with TileContext(nc) as tc:
        with (
            tc.tile_pool(name="const", bufs=1) as constp,
            tc.tile_pool(name="xp", bufs=4) as xp,
            tc.tile_pool(name="wp", bufs=1) as wp,
            tc.tile_pool(name="hp", bufs=1) as hp,
            tc.tile_pool(name="op", bufs=4) as op,
            tc.tile_pool(name="ps1", bufs=2, space="PSUM") as ps1p,
            tc.tile_pool(name="psy", bufs=1, space="PSUM") as psyp,
        ):
            zt = constp.tile([P, 2 * P], BF16, tag="zt")
            nc.vector.memset(zt[:], 0.0)
            wups = psyp.tile([P, 2, 256], FP32, tag="warm")
            for i in range(N_WARMUP):
                nc.tensor.matmul(wups[:, i % 2, :128], zt[:, :P],
                                 zt[:, :128], start=True, stop=True)

            w_sb = [wp.tile([P, WSL], BF16, tag=f"w_{sl}", name=f"w_{sl}")
                    for sl in range(NSL)]

            x_t = []

            def load_x(g):
                t = xp.tile([P, KH, CKS], BF16, tag="x", name=f"x_{g}")
                base = KH * ck0 + (g - 1) * KH * CKS
                nc.sync.dma_start(
                    out=t[:], in_=xc[:, base:base + KH * CKS])
                x_t.append(t)

            # start window: tight x chunk0 + slab(0) w1 m0 interleaved,
            # b1, rest of slab0 w1, slab1 w1 (seg0's GEMM1 set), then x c1
            x0t = constp.tile([P, KH, ck0], BF16, tag="x0", name="x_0")
            x_t.append(x0t)
            nc.sync.dma_start(out=x0t[:, :2, :], in_=xc[:, :2 * ck0])
            nc.sync.dma_start(out=w_sb[0][:, :4 * P], in_=wall[:P, :4 * P])
            nc.sync.dma_start(out=x0t[:, 2:4, :],
                              in_=xc[:, 2 * ck0:4 * ck0])
            nc.sync.dma_start(out=w_sb[0][:, 4 * P:KH * P],
                              in_=wall[:P, 4 * P:KH * P])
            nc.sync.dma_start(out=x0t[:, 4:, :],
                              in_=xc[:, 4 * ck0:KH * ck0])
            b1_sb = constp.tile([P, NSEG * FL // P], FP32, tag="b1")
            nc.sync.dma_start(out=b1_sb[:], in_=b1c[:])
            for m in range(1, MF):
                nc.sync.dma_start(
                    out=w_sb[0][:, m * KH * P:(m + 1) * KH * P],
                    in_=wall[:P, m * KH * P:(m + 1) * KH * P])
            nc.sync.dma_start(out=w_sb[1][:, :WS1],
                              in_=wall[P:2 * P, :WS1])
            if NCH > 1:
                load_x(1)
            # seg0 w2, x c2, seg1 w1, x c3, seg1 w2, then remaining
            # segs' weights (x c4+ are emitted inline in the chunk loop)
            for sl in (0, 1):
                nc.sync.dma_start(out=w_sb[sl][:, WS1:],
                                  in_=wall[sl * P:(sl + 1) * P, WS1:])
            if NCH > 2:
                load_x(2)
            for sl in (2, 3):
                nc.sync.dma_start(out=w_sb[sl][:, :WS1],
                                  in_=wall[sl * P:(sl + 1) * P, :WS1])
            if NCH > 3:
                load_x(3)
            for sl in (2, 3):
                nc.sync.dma_start(out=w_sb[sl][:, WS1:],
                                  in_=wall[sl * P:(sl + 1) * P, WS1:])
            for s in (2, 3):
                for part in range(2):
                    for sl in (2 * s, 2 * s + 1):
                        r0, r1 = sl * P, (sl + 1) * P
                        if part == 0:
                            nc.sync.dma_start(out=w_sb[sl][:, :WS1],
                                              in_=wall[r0:r1, :WS1])
                        else:
                            nc.sync.dma_start(out=w_sb[sl][:, WS1:],
                                              in_=wall[r0:r1, WS1:])

            def w1sl(sl, m, k):
                return w_sb[sl][:, m * KH * P + k * P:m * KH * P + (k + 1) * P]

            def w2sl(sl, m, ht):
                base = WS1 + m * H + ht * P
                return w_sb[sl][:, base:base + P]

            for g, (s, ci, gcoff, ck) in enumerate(chunks):
                last_chunk = g == NCH - 1

                def ytile(q, half):
                    return psyp.tile([P, 512], FP32, tag=f"y{q}",
                                     name=f"y{q}_{g}_{half}")

                def o4tile(half):
                    return op.tile([P, 4, 512], FP16, tag="o4",
                                   name=f"o4_{g}_{half}")

                # GEMM1: h for the segment's 2 slabs staged in SBUF
                hL = hp.tile([P, NTHL, MF, 512], BF16, tag="h",
                             name=f"h_{g}")
                for th in range(NTHL):
                    sl = s * NTHL + th
                    for m in range(MF):
                        pt = ps1p.tile([P, 512], FP32, tag="ps1")
                        for k in range(KH):
                            nc.tensor.matmul(
                                pt[:, :ck],
                                w1sl(sl, m, k),
                                x_t[g][:, k, :ck],
                                start=(k == 0), stop=(k == KH - 1),
                            )
                        bidx = s * (FL // P) + th * MF + m
                        nc.scalar.activation(
                            hL[:, th, m, :ck], pt[:, :ck], GELU,
                            bias=b1_sb[:, bidx:bidx + 1],
                        )

                # prefetch x for chunk g+4 (chunks 0-3 preloaded; slot
                # (g+4)%4 is chunk g's own, whose readers — this chunk's
                # GEMM1, just emitted — retire before the DMA fires)
                if g + 4 < NCH:
                    load_x(g + 4)

                for half in range(2):
                    y_q = [ytile(q, half) for q in range(4)]
                    o4 = o4tile(half)

                    def evict(q):
                        nc.vector.tensor_copy(
                            o4[:, q, :ck], y_q[q][:, :ck])

                    if last_chunk and half == 1:
                        for q in range(4):
                            ht = 4 * half + q
                            for th in range(NTHL):
                                sl = s * NTHL + th
                                for m in range(MF):
                                    nc.tensor.matmul(
                                        y_q[q][:, :ck],
                                        w2sl(sl, m, ht),
                                        hL[:, th, m, :ck],
                                        start=(th == 0 and m == 0),
                                        stop=(th == NTHL - 1 and m == MF - 1),
                                    )
                            evict(q)
                            nc.sync.dma_start(
                                out=out_v[:, ht:ht + 1, gcoff:gcoff + ck],
                                in_=o4[:, q:q + 1, :ck])
                        continue
                    else:
                        for th in range(NTHL):
                            sl = s * NTHL + th
                            for m in range(MF):
                                for q in range(4):
                                    nc.tensor.matmul(
                                        y_q[q][:, :ck],
                                        w2sl(sl, m, 4 * half + q),
                                        hL[:, th, m, :ck],
                                        start=(th == 0 and m == 0),
                                        stop=(th == NTHL - 1 and m == MF - 1),
                                    )
                        for q in range(4):
                            evict(q)
                    # mid-kernel stores ride the Act HWDGE queue: slow
                    # (~35GB/s) but idle, so they never queue behind the
                    # 25MB weight/x backlog on the SP queue — which stalls
                    # the o4-slot WAR chain into the PE (22us when stores
                    # rode SP; 4.6us even alternating h0 stores onto SP).
                    # The last chunk's stores use the by-then-empty SP
                    # queue for a fast tail.
                    eng = nc.sync if last_chunk else nc.scalar
                    eng.dma_start(
                        out=out_v[:, 4 * half:4 * half + 4, gcoff:gcoff + ck],
                        in_=o4[:, :, :ck])

    nc.compile()
    return nc


def _route(x: np.ndarray, router_w: np.ndarray):
    logits = x @ router_w.T                                   # [T, E]
    top_i = np.argsort(-logits, axis=1, kind="stable")[:, :TOPK]
    top_v = np.take_along_axis(logits, top_i, axis=1)
    mx = top_v.max(axis=1, keepdims=True)
    ex = np.exp(top_v - mx)
    rw = ex / ex.sum(axis=1, keepdims=True)
    return top_i, rw.astype(np.float32)


def _swizzle_wall_q(w1, w2, experts, j):
    """Core (quad, j)'s [NSL*P, WSL] slab matrix: per expert e (segment
    order), 2 slabs of 512 F rows from e's F-slice [j*FL,(j+1)*FL)."""
    rows = []
    for e in experts:
        w2t = w2[e].T                                   # [F, H]
        for th in range(NTHL):
            f0 = j * FL + th * 512
            a = (w1[e][f0:f0 + 512]                     # [512, H]
                 .reshape(MF, P, KH, P).transpose(3, 0, 2, 1)
                 .reshape(P, MF * KH * P))
            b = (w2t[f0:f0 + 512]                       # [512, H]
                 .reshape(MF, P, H).transpose(1, 0, 2)
                 .reshape(P, MF * H))
            rows.append(np.concatenate([a, b], axis=1))
    return np.ascontiguousarray(
        np.concatenate(rows, axis=0)).astype(ml_dtypes.bfloat16)


def kernel(hidden_states, router_w, w1, b1, w2, b2):
    hidden_states = np.ascontiguousarray(np.asarray(hidden_states, np.float32))
    router_w = np.ascontiguousarray(np.asarray(router_w, np.float32))
    w1 = np.asarray(w1, np.float32)
    b1 = np.asarray(b1, np.float32)
    w2 = np.asarray(w2, np.float32)
    b2 = np.asarray(b2, np.float32)

    B, S, _ = hidden_states.shape
    T = B * S
    x = hidden_states.reshape(T, H)

    top_i, rw = _route(x, router_w)

    sel_idx = []
    sel_gate = []
    counts = np.zeros(E, np.int64)
    for e in range(E):
        mask = top_i == e
        rows = np.nonzero(mask.any(axis=1))[0]
        g = rw[rows[:, None], np.argmax(mask[rows], axis=1)[:, None]][:, 0]
        sel_idx.append(rows)
        sel_gate.append(g.astype(np.float32))
        counts[e] = len(rows)

    # interleave sorted experts into 2 quads so rank-r loads match
    order = np.argsort(-counts, kind="stable")
    quads = [order[0::2], order[1::2]]
    segs = tuple(
        max(8, -(-int(max(counts[quads[0][r]], counts[quads[1][r]])) // 8) * 8)
        for r in range(NSEG))
    seg_off = [sum(segs[:s]) for s in range(NSEG)]

    if segs not in _cache:
        _cache[segs] = _build(segs)
    nc = _cache[segs]

    in_maps = [None] * N_CORES
    for q, experts in enumerate(quads):
        # xc shared by the quad's 4 cores: chunk 0 tight [P, KH*ck0],
        # later chunks zero-padded [P, KH*CKS] blocks, col = k*W + c
        blocks = []
        first = True
        for r in range(NSEG):
            e = experts[r]
            Ss = segs[r]
            xpad = np.zeros((Ss, H), np.float32)
            n_e = counts[e]
            xpad[:n_e] = x[sel_idx[e]]
            for coff, ck in _chunks(Ss):
                W = ck if first else CKS
                first = False
                blk = np.zeros((P, KH, W), np.float32)
                blk[:, :, :ck] = (xpad[coff:coff + ck]
                                  .reshape(ck, KH, P).transpose(2, 1, 0))
                blocks.append(blk.reshape(P, KH * W))
        xq = np.ascontiguousarray(
            np.concatenate(blocks, axis=1)).astype(ml_dtypes.bfloat16)

        for j in range(NSPLIT):
            b1j = np.concatenate(
                [b1[experts[r]][j * FL:(j + 1) * FL].reshape(FL // P, P).T
                 for r in range(NSEG)], axis=1)
            in_maps[q * NSPLIT + j] = {
                "wall": _swizzle_wall_q(w1, w2, experts, j),
                "xc": xq,
                "b1c": np.ascontiguousarray(b1j),
            }

    global LAST_EXEC_TIME_NS
    LAST_EXEC_TIME_NS = 0
    res = run_bass_kernel_spmd(nc, in_maps, list(range(N_CORES)), trace=TRACE)
    if res.exec_time_ns:
        LAST_EXEC_TIME_NS = res.exec_time_ns

    out = np.zeros((T, H), np.float32)
    for q, experts in enumerate(quads):
        for r in range(NSEG):
            e = experts[r]
            rows, g = sel_idx[e], sel_gate[e]
            if not len(rows):
                continue
            o = seg_off[r]
            y = sum(res.results[q * NSPLIT + j]["out"][:, o:o + len(rows)]
                    .astype(np.float32) for j in range(NSPLIT))
            out[rows] += g[:, None] * (y.T + b2[e][None, :])
    return out.reshape(B, S, H)


# revision 26
# speedup vs baseline: 1.0062x; 1.0062x over previous
"""MoE layer (top-2 of 8, H=1024, FFN=4096) on 8 TRN2 NeuronCores —
4-way expert-F-split for load balance.

Two quads of 4 cores; quad q serves 4 experts (experts interleaved by
sorted token count so rank-r segment sizes match across quads). Core j
of a quad holds F-rows [j*1024, (j+1)*1024) of ALL 4 of its experts
(16.8MB, same weight footprint as expert-parallel). Each core streams
the quad's full token set once per segment (4 segments, one per
expert); per-core rows = 128 * sum(S_s) with S_s = rank-r max over the
two quads. vs expert-parallel's 512*maxcount: always <=, and ~3% less
at this seed's near-balanced routing (4128 vs 4256 token-equivalents).
Host sums the 4 F-slice partials (fp16) per expert and folds b2 + the
top-2 softmax gates.

Device schedule per core: segments sequential; per segment, token
chunks of <=512 (PSUM width; chunks stay >=236 tokens — below that
matmuls go LDWEIGHTS-bound at 98ns/load). Per chunk: GEMM1 (2 slabs x
4 f-tiles x 8 k) -> gelu+b1 -> GEMM2 (8 ht x 8 accum) -> fp16 evict on
DVE -> store. x tiles rotate through 4 SBUF slots [P,8,512]; chunk 0
is a tight dedicated tile, later chunks are host-padded to the 512
slot so DMA runs are contiguous. x triggers for chunk g+4 are emitted
after chunk g's GEMM1 so the SP engine never blocks on a slot-free
wait. DMA emission order = arrival order = consumption order; the
first GEMM1 group's deps (x chunk-0 + slab-0 m0 w1) are interleaved in
~131-262KB pieces so the first real matmul starts ~10.1us (NEFF
prologue keeps DMA dead until ~8.3us; a short PE warmup covers the
p-state ramp until then). Mid-kernel output stores ride the Act HWDGE
queue (slow but idle — on the SP queue they sit FIFO behind the ~25MB
input backlog and the o4-slot WAR chain stalls the PE for 22us); the
last chunk's stores use the by-then-empty SP queue for a fast tail.

GEMMs in bf16 (fp32 4x slower; fp8 fails the 2e-2 gate per v2
measurements). Weights pre-swizzled on host to exact SBUF layout.
"""

import os

os.environ.setdefault("NEURON_RT_RESET_CORES", "1")

import ml_dtypes
import numpy as np

import concourse.bass as bass  # noqa: F401  (bass types via bacc)
import concourse.mybir as mybir
from concourse import bacc
from concourse.tile import TileContext
from concourse.bass_utils import run_bass_kernel_spmd

H = 1024
E = 8
F = 4096
TOPK = 2
P = 128
N_CORES = 8
FP32 = mybir.dt.float32
FP16 = mybir.dt.float16
BF16 = mybir.dt.bfloat16

NSEG = 4           # experts per quad / segments per core
NSPLIT = 4         # cores sharing each expert's F dim
FL = F // NSPLIT   # 1024 F rows per core per expert
NTHL = FL // 512   # 2 slabs (of 512 F) per expert per core
MF = 512 // P      # 4 f-tiles per slab
KH = H // P        # 8 contraction tiles for GEMM1
HT = H // P        # 8 output H-tiles for GEMM2
NSL = NSEG * NTHL  # 8 slabs total per core
CKS = 512          # x slot width (PSUM-limited chunk cap)

WS1 = KH * 512               # 4096 w1 cols per slab
WSL = WS1 + MF * H           # + 4096 w2 cols

# PE warmup matmuls (128 rows each): cover the p-state ramp until the
# first real GEMM1 group's DMA deps land (~10us).
N_WARMUP = 12

_cache: dict = {}

TRACE = False
LAST_EXEC_TIME_NS = None


def _chunks(C: int):
    """Near-even token chunks, multiples of 8, each <=512."""
    nch = -(-C // CKS)
    u = C // 8
    units = [u // nch + (1 if i < u % nch else 0) for i in range(nch)]
    widths = [un * 8 for un in units]
    assert sum(widths) == C and all(0 < w <= CKS for w in widths)
    out = []
    off = 0
    for w in widths:
        out.append((off, w))
        off += w
    return out


def _seg_chunks(s: int, C: int):
    """Chunks for segment s. The LAST segment ends with a 240-token
    chunk so the kernel tail (final evict + store after the last
    matmul) is half as long; 240 stays above the ~236-row threshold
    where matmuls go LDWEIGHTS-bound. Other segments near-even."""
    if s == NSEG - 1 and C >= 720:
        head = _chunks(C - 240)
        return head + [(C - 240, 240)]
    return _chunks(C)


def _build(segs: tuple):
    """Per-core program: NSEG sequential expert segments of sizes segs."""
    assert all(s % 8 == 0 and s > 0 for s in segs)
    seg_cbs = [_seg_chunks(s, segs[s]) for s in range(NSEG)]
    seg_off = [sum(segs[:s]) for s in range(NSEG)]
    CTOT = sum(segs)
    # global chunk list: (seg, chunk-in-seg, out col offset, width)
    chunks = []
    for s in range(NSEG):
        for ci, (coff, ck) in enumerate(seg_cbs[s]):
            chunks.append((s, ci, seg_off[s] + coff, ck))
    NCH = len(chunks)

    nc = bacc.Bacc("TRN2", target_bir_lowering=False, debug=False,
                   num_devices=N_CORES)

    ck0 = chunks[0][3]
    BU = ck0 + P   # boot unit width per k: x0 k-slice + w1 slab0 m0 k-slice
    wall = nc.dram_tensor("wall", [NSL * P, WSL], BF16, kind="ExternalInput")
    # boot: the start-window deps (x chunk 0 + slab-0 m0 w1) interleaved
    # BY K-SLICE in one contiguous layout, so the first DMAs have ~3.9KB
    # per-partition runs (the separate 1-1.4KB-run x0/w1m0 pieces only
    # reached ~0.25MB/us) and arrival order = consumption order exactly
    boot = nc.dram_tensor("boot", [P, KH * BU], BF16, kind="ExternalInput")
    # x chunks 1+: [P, KH*CKS] blocks (cols k*CKS + c, zero-padded past
    # ck) so slot DMA is fully contiguous
    xc = nc.dram_tensor("xc", [P, (NCH - 1) * KH * CKS], BF16,
                        kind="ExternalInput")
    b1c = nc.dram_tensor("b1c", [P, NSEG * FL // P], FP32,
                         kind="ExternalInput")
    out = nc.dram_tensor("out", [H, CTOT], FP16, kind="ExternalOutput")

    out_v = out.rearrange("(t p) c -> p t c", p=P)   # [128, 8, CTOT]

    GELU = mybir.ActivationFunctionType.Gelu

    with TileContext(nc) as tc:
        with (
            tc.tile_pool(name="const", bufs=1) as constp,
            tc.tile_pool(name="xp", bufs=4) as xp,
            tc.tile_pool(name="wp", bufs=1) as wp,
            tc.tile_pool(name="hp", bufs=1) as hp,
            tc.tile_pool(name="op", bufs=4) as op,
            tc.tile_pool(name="ps1", bufs=2, space="PSUM") as ps1p,
            tc.tile_pool(name="psy", bufs=1, space="PSUM") as psyp,
        ):
            zt = constp.tile([P, 2 * P], BF16, tag="zt")
            nc.vector.memset(zt[:], 0.0)
            wups = psyp.tile([P, 2, 256], FP32, tag="warm")
            for i in range(N_WARMUP):
                nc.tensor.matmul(wups[:, i % 2, :128], zt[:, :P],
                                 zt[:, :128], start=True, stop=True)

            w_sb = [wp.tile([P, WSL], BF16, tag=f"w_{sl}", name=f"w_{sl}")
                    for sl in range(NSL)]

            x_t = []

            def load_x(g):
                t = xp.tile([P, KH, CKS], BF16, tag="x", name=f"x_{g}")
                base = (g - 1) * KH * CKS
                nc.sync.dma_start(
                    out=t[:], in_=xc[:, base:base + KH * CKS])
                x_t.append(t)

            # start window: the boot block in 3 k-ordered pieces (k0-2,
            # k3-5, k6-7), then b1, rest of slab0 w1, slab1 w1 (seg0's
            # GEMM1 set), then x c1
            boot_t = constp.tile([P, KH * BU], BF16, tag="boot",
                                 name="boot")
            x_t.append(None)   # chunk 0's x lives in boot_t
            nc.sync.dma_start(out=boot_t[:, :3 * BU], in_=boot[:, :3 * BU])
            nc.sync.dma_start(out=boot_t[:, 3 * BU:6 * BU],
                              in_=boot[:, 3 * BU:6 * BU])
            nc.sync.dma_start(out=boot_t[:, 6 * BU:], in_=boot[:, 6 * BU:])
            b1_sb = constp.tile([P, NSEG * FL // P], FP32, tag="b1")
            nc.sync.dma_start(out=b1_sb[:], in_=b1c[:])
            for m in range(1, MF):
                nc.sync.dma_start(
                    out=w_sb[0][:, m * KH * P:(m + 1) * KH * P],
                    in_=wall[:P, m * KH * P:(m + 1) * KH * P])
            nc.sync.dma_start(out=w_sb[1][:, :WS1],
                              in_=wall[P:2 * P, :WS1])
            if NCH > 1:
                load_x(1)
            # seg0 w2, x c2, seg1 w1, x c3, seg1 w2, then remaining
            # segs' weights (x c4+ are emitted inline in the chunk loop)
            for sl in (0, 1):
                nc.sync.dma_start(out=w_sb[sl][:, WS1:],
                                  in_=wall[sl * P:(sl + 1) * P, WS1:])
            if NCH > 2:
                load_x(2)
            for sl in (2, 3):
                nc.sync.dma_start(out=w_sb[sl][:, :WS1],
                                  in_=wall[sl * P:(sl + 1) * P, :WS1])
            if NCH > 3:
                load_x(3)
            for sl in (2, 3):
                nc.sync.dma_start(out=w_sb[sl][:, WS1:],
                                  in_=wall[sl * P:(sl + 1) * P, WS1:])
            for s in (2, 3):
                for part in range(2):
                    for sl in (2 * s, 2 * s + 1):
                        r0, r1 = sl * P, (sl + 1) * P
                        if part == 0:
                            nc.sync.dma_start(out=w_sb[sl][:, :WS1],
                                              in_=wall[r0:r1, :WS1])
                        else:
                            nc.sync.dma_start(out=w_sb[sl][:, WS1:],
                                              in_=wall[r0:r1, WS1:])

            def w1sl(sl, m, k):
                if sl == 0 and m == 0:
                    return boot_t[:, k * BU + ck0:(k + 1) * BU]
                return w_sb[sl][:, m * KH * P + k * P:m * KH * P + (k + 1) * P]

            def xap(g, k, ck):
                if g == 0:
                    return boot_t[:, k * BU:k * BU + ck]
                return x_t[g][:, k, :ck]

            def w2sl(sl, m, ht):
                base = WS1 + m * H + ht * P
                return w_sb[sl][:, base:base + P]

            for g, (s, ci, gcoff, ck) in enumerate(chunks):
                last_chunk = g == NCH - 1

                def ytile(q, half):
                    return psyp.tile([P, 512], FP32, tag=f"y{q}",
                                     name=f"y{q}_{g}_{half}")

                def o4tile(half):
                    return op.tile([P, 4, 512], FP16, tag="o4",
                                   name=f"o4_{g}_{half}")

                # GEMM1: h for the segment's 2 slabs staged in SBUF
                hL = hp.tile([P, NTHL, MF, 512], BF16, tag="h",
                             name=f"h_{g}")
                for th in range(NTHL):
                    sl = s * NTHL + th
                    for m in range(MF):
                        pt = ps1p.tile([P, 512], FP32, tag="ps1")
                        for k in range(KH):
                            nc.tensor.matmul(
                                pt[:, :ck],
                                w1sl(sl, m, k),
                                xap(g, k, ck),
                                start=(k == 0), stop=(k == KH - 1),
                            )
                        bidx = s * (FL // P) + th * MF + m
                        nc.scalar.activation(
                            hL[:, th, m, :ck], pt[:, :ck], GELU,
                            bias=b1_sb[:, bidx:bidx + 1],
                        )

                # prefetch x for chunk g+4 (chunks 0-3 preloaded; slot
                # (g+4)%4 is chunk g's own, whose readers — this chunk's
                # GEMM1, just emitted — retire before the DMA fires)
                if g + 4 < NCH:
                    load_x(g + 4)

                for half in range(2):
                    y_q = [ytile(q, half) for q in range(4)]
                    o4 = o4tile(half)

                    def evict(q):
                        nc.vector.tensor_copy(
                            o4[:, q, :ck], y_q[q][:, :ck])

                    if last_chunk and half == 1:
                        for q in range(4):
                            ht = 4 * half + q
                            for th in range(NTHL):
                                sl = s * NTHL + th
                                for m in range(MF):
                                    nc.tensor.matmul(
                                        y_q[q][:, :ck],
                                        w2sl(sl, m, ht),
                                        hL[:, th, m, :ck],
                                        start=(th == 0 and m == 0),
                                        stop=(th == NTHL - 1 and m == MF - 1),
                                    )
                            evict(q)
                            nc.sync.dma_start(
                                out=out_v[:, ht:ht + 1, gcoff:gcoff + ck],
                                in_=o4[:, q:q + 1, :ck])
                        continue
                    else:
                        for th in range(NTHL):
                            sl = s * NTHL + th
                            for m in range(MF):
                                for q in range(4):
                                    nc.tensor.matmul(
                                        y_q[q][:, :ck],
                                        w2sl(sl, m, 4 * half + q),
                                        hL[:, th, m, :ck],
                                        start=(th == 0 and m == 0),
                                        stop=(th == NTHL - 1 and m == MF - 1),
                                    )
                        for q in range(4):
                            evict(q)
                    # mid-kernel stores ride the Act HWDGE queue: slow
                    # (~35GB/s) but idle, so they never queue behind the
                    # 25MB weight/x backlog on the SP queue — which stalls
                    # the o4-slot WAR chain into the PE (22us when stores
                    # rode SP; 4.6us even alternating h0 stores onto SP).
                    # The last chunk's stores use the by-then-empty SP
                    # queue for a fast tail.
                    eng = nc.sync if last_chunk else nc.scalar
                    eng.dma_start(
                        out=out_v[:, 4 * half:4 * half + 4, gcoff:gcoff + ck],
                        in_=o4[:, :, :ck])

    nc.compile()
    return nc


def _route(x: np.ndarray, router_w: np.ndarray):
    logits = x @ router_w.T                                   # [T, E]
    top_i = np.argsort(-logits, axis=1, kind="stable")[:, :TOPK]
    top_v = np.take_along_axis(logits, top_i, axis=1)
    mx = top_v.max(axis=1, keepdims=True)
    ex = np.exp(top_v - mx)
    rw = ex / ex.sum(axis=1, keepdims=True)
    return top_i, rw.astype(np.float32)


def _swizzle_wall_q(w1, w2, experts, j):
    """Core (quad, j)'s [NSL*P, WSL] slab matrix: per expert e (segment
    order), 2 slabs of 512 F rows from e's F-slice [j*FL,(j+1)*FL)."""
    rows = []
    for e in experts:
        w2t = w2[e].T                                   # [F, H]
        for th in range(NTHL):
            f0 = j * FL + th * 512
            a = (w1[e][f0:f0 + 512]                     # [512, H]
                 .reshape(MF, P, KH, P).transpose(3, 0, 2, 1)
                 .reshape(P, MF * KH * P))
            b = (w2t[f0:f0 + 512]                       # [512, H]
                 .reshape(MF, P, H).transpose(1, 0, 2)
                 .reshape(P, MF * H))
            rows.append(np.concatenate([a, b], axis=1))
    return np.ascontiguousarray(
        np.concatenate(rows, axis=0)).astype(ml_dtypes.bfloat16)


def kernel(hidden_states, router_w, w1, b1, w2, b2):
    hidden_states = np.ascontiguousarray(np.asarray(hidden_states, np.float32))
    router_w = np.ascontiguousarray(np.asarray(router_w, np.float32))
    w1 = np.asarray(w1, np.float32)
    b1 = np.asarray(b1, np.float32)
    w2 = np.asarray(w2, np.float32)
    b2 = np.asarray(b2, np.float32)

    B, S, _ = hidden_states.shape
    T = B * S
    x = hidden_states.reshape(T, H)

    top_i, rw = _route(x, router_w)

    sel_idx = []
    sel_gate = []
    counts = np.zeros(E, np.int64)
    for e in range(E):
        mask = top_i == e
        rows = np.nonzero(mask.any(axis=1))[0]
        g = rw[rows[:, None], np.argmax(mask[rows], axis=1)[:, None]][:, 0]
        sel_idx.append(rows)
        sel_gate.append(g.astype(np.float32))
        counts[e] = len(rows)

    # interleave sorted experts into 2 quads so rank-r loads match
    order = np.argsort(-counts, kind="stable")
    quads = [order[0::2], order[1::2]]
    segs = tuple(
        max(8, -(-int(max(counts[quads[0][r]], counts[quads[1][r]])) // 8) * 8)
        for r in range(NSEG))
    seg_off = [sum(segs[:s]) for s in range(NSEG)]

    if segs not in _cache:
        _cache[segs] = _build(segs)
    nc = _cache[segs]

    in_maps = [None] * N_CORES
    for q, experts in enumerate(quads):
        # xc shared by the quad's 4 cores: chunks 1+ as zero-padded
        # [P, KH*CKS] blocks (col = k*CKS + c). Chunk 0 is tight and
        # goes into the per-core boot tensor, interleaved by k-slice
        # with slab-0 m0 w1.
        blocks = []
        x0blk = None
        for r in range(NSEG):
            e = experts[r]
            Ss = segs[r]
            xpad = np.zeros((Ss, H), np.float32)
            n_e = counts[e]
            xpad[:n_e] = x[sel_idx[e]]
            for coff, ck in _seg_chunks(r, Ss):
                W = ck if x0blk is None else CKS
                blk = np.zeros((P, KH, W), np.float32)
                blk[:, :, :ck] = (xpad[coff:coff + ck]
                                  .reshape(ck, KH, P).transpose(2, 1, 0))
                if x0blk is None:
                    x0blk = blk            # [P, KH, ck0]
                else:
                    blocks.append(blk.reshape(P, KH * W))
        xq = np.ascontiguousarray(
            np.concatenate(blocks, axis=1)).astype(ml_dtypes.bfloat16)
        x0blk = x0blk.astype(ml_dtypes.bfloat16)

        for j in range(NSPLIT):
            b1j = np.concatenate(
                [b1[experts[r]][j * FL:(j + 1) * FL].reshape(FL // P, P).T
                 for r in range(NSEG)], axis=1)
            wallj = _swizzle_wall_q(w1, w2, experts, j)
            bootj = np.concatenate(
                [np.concatenate(
                    [x0blk[:, k, :], wallj[:P, k * P:(k + 1) * P]], axis=1)
                 for k in range(KH)], axis=1)
            in_maps[q * NSPLIT + j] = {
                "wall": wallj,
                "xc": xq,
                "boot": np.ascontiguousarray(bootj),
                "b1c": np.ascontiguousarray(b1j),
            }

    global LAST_EXEC_TIME_NS
    LAST_EXEC_TIME_NS = 0
    res = run_bass_kernel_spmd(nc, in_maps, list(range(N_CORES)), trace=TRACE)
    if res.exec_time_ns:
        LAST_EXEC_TIME_NS = res.exec_time_ns

    out = np.zeros((T, H), np.float32)
    for q, experts in enumerate(quads):
        for r in range(NSEG):
            e = experts[r]
            rows, g = sel_idx[e], sel_gate[e]
            if not len(rows):
                continue
            o = seg_off[r]
            y = sum(res.results[q * NSPLIT + j]["out"][:, o:o + len(rows)]
                    .astype(np.float32) for j in range(NSPLIT))
            out[rows] += g[:, None] * (y.T + b2[e][None, :])
    return out.reshape(B, S, H)
